# revision 24
# baseline (speedup 1.0000x reference)
"""Directional contrastive loss on 8 Trainium2 NeuronCores.

Math: with all labels equal (per the problem spec) the mask is all-ones and

  loss = mean_{n,i,j} log(denom + 1e-6)        ... (over N*H*W)
         - mean_{n,m,i,j} logits               ... (over N*M*H*W)

  logits[n,m,i,j] = <fn[n,:,i,j], fn[n,:, i+d0[m,i,j], j+d1[m,i,j]]> / T
  denom[n,i,j]    = sum_m exp(logits[n,m,i,j])

Since (d0,d1) in {-1,0,1}^2, logits take at most 9 values per (n,i,j):
S_k[n,i,j] = cos(x[n,:,i,j], x[n,:,i+di,j+dj]) / T for the 9 offsets k.
With cnt_k[i,j] = #{m : dir_m(i,j) == k} (host-precomputed from the int32
`directions` tensor):

  denom = sum_k cnt_k * exp(S_k); the self term k=(0,0) is exactly
  exp(1/T) (cos = 1), folded into a host constant.

The logit-sum side is dominated by the self terms (cos = 1 -> 1/T exactly),
which the host computes exactly from cnt. The non-self logits are zero-mean
cos values (C=192 random normals, |cos| ~ 1/sqrt(192)); their sum over
~800k samples contributes ~1e-3 absolute (~2e-4 relative) to the loss and
is dropped on purpose (tolerance is 2e-2 relative).

Sharding: by batch - core n owns batch n (the cross-batch coupling lives
entirely in the tiny replicated cnt maps, so no halos are needed).  Per core
the kernel computes 4 shifted correlation maps R_k = sum_c x*shift_k(x) (the
other 4 follow by symmetry R_{-k}[p] = R_k[p - k]) plus the self map
R_00 = sum_c x^2 used for the normalization.  Products run on DVE in bf16;
the channel reduction runs on the tensor engine as selector-column matmuls
accumulating into psum partition rows; squares/exp/log on the scalar
engine.  All scalar-engine funcs (Square/Exp/Ln/Copy) live in the single
`natural_log_exp_and_others` activation table set, so the table loads once
and never thrashes; 1/sqrt(R00) is computed as Exp(-0.5*Ln(R00)).
Stage 2 runs in bf16 so the DVE ops hit the 2x packed perf mode.
Each core returns per-partition (= per image row) log-denom row sums in a
[128, 1] tensor; the host adds them up, adds the exact self-logit term and
scales.
"""

import os
import sys

import numpy as np

for _p in ("/opt/trn_rl_repo", "/root/.axon_site/_ro/trn_rl_repo"):
    if os.path.isdir(_p) and _p not in sys.path:
        sys.path.insert(0, _p)

import contextlib

import concourse.bacc as bacc
import concourse.mybir as mybir
from concourse import tile
from concourse.bass_utils import run_bass_kernel_spmd

from ml_dtypes import bfloat16

# Force every activation onto the one table set that holds all functions
# this kernel uses (Square/Exp/Ln/Copy): hide the other sets from the
# table-load pass (positions preserved, so the emitted act_func_set_id
# still indexes the real act_info.json). Without this the pass alternates
# exp_and_others <-> natural_log, reloading tables ~2x per rep (~2.6us
# of scalar-engine time and a longer serial tail).
from concourse.hw_specs import get_activation_tables as _gat_orig


def _gat_single_set(arch):
    d = dict(_gat_orig(arch))
    keep = "natural_log_exp_and_others"
    if keep in d:
        d = {k: (v if k == keep else set()) for k, v in d.items()}
    return d


bacc.get_activation_tables = _gat_single_set

N, C, H, W = 8, 192, 112, 112
TEMP = 0.1
CORES = 8                # core n owns batch n (no spatial halos needed)
PIX = H * W              # 12544 pixels per core
X2R = 57                 # rows per x2 half (56 owned + 1 partner row)
X2W = X2R * W            # 6384
PAD = 128                # column padding on the packed feature tiles
CH = 448                 # psum chunk (4 partition-rows x 112)
NQ = PIX // CH           # 28 chunks per map
NCH_CM = 9               # constant-map channels
X1BLKS = [(0, 1792), (1792, 3584), (5376, 3584), (8960, 3584)]

_dt = mybir.dt
_F32 = _dt.float32
_BF16 = _dt.bfloat16

# shift offsets in pixel-linear space for maps m=0..4:
# 0: self (0,0), 1: (0,+1), 2: (+1,-1), 3: (+1,0), 4: (+1,+1)
DELTAS = [0, 1, W - 1, W, W + 1]


def _cap(base, dims, off):
    """Custom access pattern: keep base's partition dim, replace the free
    dims with `dims` ([stride, count] outer->inner) at element offset `off`."""
    import bass_rust
    return bass_rust.AP(tensor=base.tensor, offset=base.offset + off,
                        ap=[list(base.ap[0])] + [list(d) for d in dims])


class Tiles:
    """Constant + persistent tiles allocated once, shared by all reps."""


def alloc_tiles(nc, ctx, tc):
    AF = mybir.ActivationFunctionType
    t = Tiles()
    t.mp = ctx.enter_context(tc.tile_pool(name="mainp", bufs=1))
    t.pp = ctx.enter_context(tc.tile_pool(name="prodp", bufs=10))
    t.sp = ctx.enter_context(tc.tile_pool(name="s2p", bufs=1))
    t.qp = ctx.enter_context(tc.tile_pool(name="psump", bufs=1, space="PSUM"))
    mp = t.mp

    t.x1t = mp.tile([128, PAD + PIX + PAD], _BF16, tag="x1t")
    t.x2t = mp.tile([128, PAD + X2W + PAD], _BF16, tag="x2t")
    t.cmt = mp.tile([128, NCH_CM * W], _BF16, tag="cmt")
    # Stationary selector banks: Z*[*, 31-r:63-r] puts the selector
    # column at position r of a [128, 32] lhsT, zeros elsewhere, so an
    # M=32 matmul accumulates one result row into psum row r of a
    # quadrant while adding 0 to the other 31 rows.
    t.z_ones = mp.tile([128, 63], _BF16, tag="z_ones")
    # z2 carries TWO selector columns 14 apart: window r puts the
    # upper-half selector at column r (psum row r, pixel half A) and the
    # lower-half selector at column r+14 (psum row r+14, pixel half B),
    # so one matmul folds an x2 product chunk into both pixel halves.
    t.z2 = mp.tile([128, 63], _BF16, tag="z2")
    t.R0 = mp.tile([128, 114], _BF16, tag="R0")
    t.R4 = mp.tile([128, 4, 114], _BF16, tag="R4")   # maps 1..4 stacked
    t.S4 = mp.tile([128, 4, 114], _BF16, tag="S4")
    # rnx[p, 0, c] = rn10[p, c+1]; rnx[p, k, c] = rn10[p+1, c+k-2] (k=1..3):
    # the per-map neighbor-side 1/(T*norm) windows, so S4 = t4 * rnx is a
    # single DVE op over all 4 maps.
    t.rnx = mp.tile([128, 4, 114], _BF16, tag="rnx")
    t.DvDs = mp.tile([128, W], _BF16, tag="DvDs")
    t.biaseps = mp.tile([128, 1], _F32, tag="biaseps")
    t.lnT = mp.tile([128, 1], _F32, tag="lnT")       # ln(1/TEMP)

    for z in (t.z_ones, t.z2):
        nc.gpsimd.memset(z[:], 0.0)
    nc.gpsimd.memset(t.z_ones[:, 31:32], 1.0)
    nc.gpsimd.memset(t.z2[0:64, 31:32], 1.0)
    nc.gpsimd.memset(t.z2[64:128, 45:46], 1.0)
    # R00 pads are 1.0 (rn=1 there; Ln's spline table mishandles huge
    # inputs like 1e30). Every pad/wrapped contribution is killed by a
    # zero cnt weight, not by rn, so rn=1 at pads is safe.
    nc.gpsimd.memset(t.R0[:], 1.0)
    nc.gpsimd.memset(t.R4[:], 0.0)
    nc.gpsimd.memset(t.S4[:], 0.0)
    nc.gpsimd.memset(t.rnx[:], 0.0)
    nc.gpsimd.memset(t.DvDs[0:1, :], 0.0)
    nc.gpsimd.memset(t.biaseps[:], 1e-6)
    nc.gpsimd.memset(t.lnT[:], float(np.log(1.0 / TEMP)))
    return t


def emit_kernel(nc, t, x1d, x2d, cmd, outd):
    AF = mybir.ActivationFunctionType
    OP = mybir.AluOpType
    mp, pp, sp, qp = t.mp, t.pp, t.sp, t.qp
    x1t, x2t, cmt = t.x1t, t.x2t, t.cmt
    z_ones, z2, R0, R4, S4, rnx = t.z_ones, t.z2, t.R0, t.R4, t.S4, t.rnx

    # Load order: x1 block 0 and all of x2 first (their products unlock
    # first), then the remaining x1 blocks; cm (stage-2 only) last.
    # Chunk boundaries sit +PAD past block starts so each block's
    # shifted reads (up to +113 columns) stay within issued chunks.
    def x1_chunk(b):
        px0, npx = X1BLKS[b]
        c0, c1 = PAD + px0 + PAD, PAD + px0 + npx + PAD
        c1 = min(c1, PAD + PIX + PAD)
        if b == len(X1BLKS) - 1:
            c1 = PAD + PIX + PAD
        nc.sync.dma_start(out=x1t[:, c0:c1], in_=x1d[:, c0:c1])

    nc.sync.dma_start(out=x1t[:, 0:PAD + PAD], in_=x1d[:, 0:PAD + PAD])
    x1_chunk(0)
    nc.sync.dma_start(out=x2t[:, 0:(PAD + X2W + PAD) // 2],
                      in_=x2d[:, 0:(PAD + X2W + PAD) // 2])
    nc.sync.dma_start(out=x2t[:, (PAD + X2W + PAD) // 2:PAD + X2W + PAD],
                      in_=x2d[:, (PAD + X2W + PAD) // 2:PAD + X2W + PAD])
    x1_chunk(1)
    x1_chunk(2)
    x1_chunk(3)
    nc.sync.dma_start(out=cmt[:], in_=cmd[:])

    # ---- stage 1: correlation maps ----
    # matmul results are stacked into psum partition rows:
    #   ptA row (m-1)*32 + q for maps 1..4, ptB row q for the self map,
    # where q = 0..27 indexes the per-map 448-pixel (4-row) chunks.
    # x2 products pack pixel half A (rows 0..55) on partitions 0..63 and
    # half B (rows 56..111) on 64..127; one z2 matmul accumulates chunk r
    # of half A into psum row r and chunk r of half B into row r+14.
    ptA = qp.tile([128, CH], _F32, tag="psA")
    ptB = qp.tile([32, CH], _F32, tag="psB")

    def quad(m):
        if m == 0:
            return ptB[0:32, :], (0, 0)
        return ptA[(m - 1) * 32:m * 32, :], (0, (m - 1) * 32)

    def x1_products(px0, npx, tag, bufs):
        """One product op per map over x1 pixels [px0, px0+npx)."""
        s = PAD + px0
        out = {}
        for m in range(5):
            d = DELTAS[m]
            tt = pp.tile([128, npx], _BF16, tag=tag, name=tag, bufs=bufs)
            if m == 0:
                nc.scalar.activation(out=tt[:], in_=x1t[:, s:s + npx],
                                     func=AF.Square)
            else:
                nc.vector.tensor_tensor(out=tt[:], in0=x1t[:, s:s + npx],
                                        in1=x1t[:, s + d:s + d + npx],
                                        op=OP.mult)
            out[m] = tt
        return out

    def x1_mms(px0, npx, prods):
        for m in range(5):
            dst, tpos = quad(m)
            for c in range(npx // CH):
                q = px0 // CH + c
                nc.tensor.matmul(dst, z_ones[:, 31 - q:63 - q],
                                 prods[m][:, c * CH:(c + 1) * CH],
                                 start=(q == 0),
                                 stop=False,
                                 tile_position=tpos,
                                 skip_group_check=True)

    # Product order: x1 head, then the big x1 tail, then x2 LAST — so the
    # next rep's x1t (and x2t) input DMAs overlap the long product
    # stretches instead of stalling the DVE at the rep boundary.
    B0 = X1BLKS[0][1]                 # 1792-px head block (unlocks PE early)
    pr0 = x1_products(0, B0, "prodS", 5)
    x1_mms(0, B0, pr0)
    # big product op per map over the remaining 10752 x1 pixels
    prB = x1_products(B0, PIX - B0, "prodB", 4)
    x1_mms(B0, PIX - B0, prB)
    # x2: one product op per map over the consumed 6272 columns
    p2 = {}
    X2U = 14 * CH
    for m in range(5):
        d = DELTAS[m]
        tt = pp.tile([128, X2U], _BF16, tag="prod2", name="prod2", bufs=4)
        s = PAD
        if m == 0:
            nc.scalar.activation(out=tt[:], in_=x2t[:, s:s + X2U],
                                 func=AF.Square)
        else:
            nc.vector.tensor_tensor(out=tt[:], in0=x2t[:, s:s + X2U],
                                    in1=x2t[:, s + d:s + d + X2U],
                                    op=OP.mult)
        p2[m] = tt
    for m in range(5):
        dst, tpos = quad(m)
        for r in range(14):
            nc.tensor.matmul(dst, z2[:, 31 - r:63 - r],
                             p2[m][:, r * CH:(r + 1) * CH],
                             start=False, stop=(r == 13),
                             tile_position=tpos,
                             skip_group_check=True)

    # evacuate psum per quadrant -> staging (bf16), then scatter into R0/R4
    stA = mp.tile([128, CH], _BF16, tag="stA")
    stB = mp.tile([32, CH], _BF16, tag="stB")
    for qd in range(4):
        nc.scalar.activation(out=stA[qd * 32:qd * 32 + NQ, :],
                             in_=ptA[qd * 32:qd * 32 + NQ, :], func=AF.Copy)
    nc.scalar.activation(out=stB[0:NQ, :], in_=ptB[0:NQ, :], func=AF.Copy)
    nc.sync.dma_start(out=R0[0:112, 1:113], in_=stB[0:NQ, :])
    for m in range(1, 5):
        nc.sync.dma_start(out=R4[0:112, m - 1, 1:113],
                          in_=stA[(m - 1) * 32:(m - 1) * 32 + NQ, :])

    # ---- stage 2: softmax-style assembly (bf16, fused stacked-map ops) ----
    # ||x||^2 >= ~80 for this data (random normals, C=192), so the
    # reference's max(norm, 1e-12) clamp is an identity and is skipped.
    # rn = 1/||x|| = Exp(-0.5*Ln(R00)); rn10 = rn/TEMP (bias = ln(1/T)).
    # Using Exp/Ln instead of Sqrt keeps every activation in the
    # natural_log_exp_and_others table set (no per-rep table reloads).
    Lr = mp.tile([128, 114], _F32, tag="Lr")
    nc.scalar.activation(out=Lr[:], in_=R0[:], func=AF.Ln)
    rn = mp.tile([128, 114], _BF16, tag="rn")
    nc.scalar.activation(out=rn[:], in_=Lr[:], func=AF.Exp, scale=-0.5)
    rn10 = mp.tile([128, 114], _BF16, tag="rn10")
    nc.scalar.activation(out=rn10[:], in_=Lr[:], func=AF.Exp, scale=-0.5,
                         bias=t.lnT[:])

    # rnx plane 0 = rn10 shifted one column (same-row neighbor); planes 1-3
    # = rn10 shifted one partition (next image row) and -1/0/+1 columns.
    # Engine APs can't shift partitions, so planes 1-3 are one SBUF->SBUF
    # DMA re-reading rn10[1:128] at three overlapping column windows.
    nc.scalar.activation(out=rnx[:, 0, 0:113], in_=rn10[:, 1:114],
                         func=AF.Copy)
    nc.sync.dma_start(out=rnx[0:127, 1:4, 1:113],
                      in_=_cap(rn10[1:128, :], [[1, 3], [1, 112]], 0))

    # S4[:, i, c] = logits map for shift i (maps (0,1),(1,-1),(1,0),(1,1)),
    # col c = pixel j+1.  t4 = R4 * rn broadcast over the map axis; then
    # one fused op multiplies all 4 maps by their neighbor windows.
    t4 = sp.tile([128, 4, 114], _BF16, tag="t4")
    nc.vector.tensor_tensor(out=t4[:], in0=R4[:],
                            in1=rn[:].unsqueeze(1).broadcast_to((128, 4, 114)),
                            op=OP.mult)
    nc.vector.tensor_tensor(out=S4[:, 0:4, 1:113], in0=_cap(t4[:], [[114, 4], [1, 112]], 1),
                            in1=_cap(rnx[:], [[114, 4], [1, 112]], 1),
                            op=OP.mult)
    E4 = mp.tile([128, 4, 114], _BF16, tag="E4")
    nc.scalar.activation(out=E4[:], in_=S4[:], func=AF.Exp)

    # contribution source APs (map axis outer, W inner so DVE stays in the
    # 2x packed mode), element order (stacked-entry, j):
    #  direct: maps 1..4 at col j+1, cnt channels 1,3,5,7
    #  derived (at source partition): maps 2,3,4 at cols j+2,j+1,j,
    #          cnt channels 4,6,8 (host-pre-shifted one row up)
    #  single: map 1 at col j (the (0,-1) term), channel 2
    cdir = _cap(cmt[:], [[2 * W, 4], [1, W]], W)
    edir = _cap(E4[:], [[114, 4], [1, W]], 1)
    cder = _cap(cmt[:], [[2 * W, 3], [1, W]], 4 * W)
    eder = _cap(E4[:], [[113, 3], [1, W]], 116)
    c3 = cmt[:, 2 * W:3 * W]
    e3 = E4[:, 0, 0:W]

    # denominator: Dv = den0 + sum_k cnt_k * E_k (direct+single), plus the
    # derived contributions computed at their source partition and shifted
    # one row down via DMA.
    tdD = sp.tile([128, 4, W], _BF16, tag="tdD")
    nc.vector.tensor_tensor(out=tdD[:], in0=cdir, in1=edir, op=OP.mult)
    u01 = sp.tile([128, 2, W], _BF16, tag="u01")
    nc.vector.tensor_tensor(out=u01[:], in0=tdD[:, 0:2, :], in1=tdD[:, 2:4, :],
                            op=OP.add)
    t1D = sp.tile([128, W], _BF16, tag="t1D")
    nc.vector.tensor_tensor(out=t1D[:], in0=c3, in1=e3, op=OP.mult)
    u2 = sp.tile([128, W], _BF16, tag="u2")
    nc.vector.tensor_tensor(out=u2[:], in0=t1D[:], in1=cmt[:, 0:W], op=OP.add)
    u3 = sp.tile([128, W], _BF16, tag="u3")
    nc.vector.tensor_tensor(out=u3[:], in0=u01[:, 0, :], in1=u01[:, 1, :],
                            op=OP.add)
    Dv = mp.tile([128, W], _BF16, tag="Dv")
    nc.vector.tensor_tensor(out=Dv[:], in0=u2[:], in1=u3[:], op=OP.add)

    tvD = sp.tile([128, 3, W], _BF16, tag="tvD")
    nc.vector.tensor_tensor(out=tvD[:], in0=cder, in1=eder, op=OP.mult)
    v0 = sp.tile([128, W], _BF16, tag="v0")
    nc.vector.tensor_tensor(out=v0[:], in0=tvD[:, 0, :], in1=tvD[:, 1, :],
                            op=OP.add)
    DvD = mp.tile([128, W], _BF16, tag="DvD")
    nc.vector.tensor_tensor(out=DvD[:], in0=v0[:], in1=tvD[:, 2, :], op=OP.add)
    nc.sync.dma_start(out=t.DvDs[1:128, :], in_=DvD[0:127, :])
    nc.vector.tensor_tensor(out=Dv[:], in0=Dv[:], in1=t.DvDs[:], op=OP.add)

    # logit-sum side: cnt-weighted sums of S, fused multiply+reduce on DVE.
    # The self-logit part (cos=1 -> 1/T) is a host constant.
    sdir = _cap(S4[:], [[114, 4], [1, W]], 1)
    sder = _cap(S4[:], [[113, 3], [1, W]], 116)
    s3 = S4[:, 0, 0:W]
    # (the free-dim reductions ride the scalar engine's activation
    # accumulator so the DVE only does the multiplies)
    outsb = mp.tile([128, 4], _F32, tag="outsb")
    sjk = sp.tile([128, 4, W], _BF16, tag="sjk")
    tdS = sp.tile([128, 4, W], _BF16, tag="tdS")
    nc.vector.tensor_tensor(out=tdS[:], in0=cdir, in1=sdir, op=OP.mult)
    nc.scalar.activation(out=sjk[:], in_=tdS[:], func=AF.Copy,
                         accum_out=outsb[:, 1:2])
    t1S = sp.tile([128, W], _BF16, tag="t1S")
    nc.vector.tensor_tensor(out=t1S[:], in0=c3, in1=s3, op=OP.mult)
    nc.scalar.activation(out=sjk[:, 0, :], in_=t1S[:], func=AF.Copy,
                         accum_out=outsb[:, 2:3])
    tvS = sp.tile([128, 3, W], _BF16, tag="tvS")
    nc.vector.tensor_tensor(out=tvS[:], in0=cder, in1=sder, op=OP.mult)
    nc.scalar.activation(out=sjk[:, 0:3, :], in_=tvS[:], func=AF.Copy,
                         accum_out=outsb[:, 3:4])

    lgd = sp.tile([128, W], _BF16, tag="s2u", name="lgd")
    nc.scalar.activation(out=lgd[:], in_=Dv[:], func=AF.Ln, bias=t.biaseps[:],
                         accum_out=outsb[:, 0:1])
    nc.sync.dma_start(out=outd[:], in_=outsb[:])
    if t.dbg is not None:
        d = t.dbg
        nc.sync.dma_start(out=d["Lr"][:], in_=Lr[:])
        nc.sync.dma_start(out=d["rn"][:], in_=rn[:])
        nc.sync.dma_start(out=d["R0"][:], in_=R0[:])
        nc.sync.dma_start(out=d["R4"][:], in_=R4[:])
        nc.sync.dma_start(out=d["S4"][:], in_=S4[:])
        nc.sync.dma_start(out=d["E4"][:], in_=E4[:])
        nc.sync.dma_start(out=d["Dv"][:], in_=Dv[:])


DEBUG_DUMP = bool(os.environ.get("KERNEL_DEBUG_DUMP"))


def build_nc(reps=1):
    """Build + compile the SPMD program. reps>1 unrolls the whole body for
    device-side timing (amortizes the axon tunnel round-trip)."""
    nc = bacc.Bacc("TRN2", target_bir_lowering=False, debug=False,
                   num_devices=CORES)
    x1d = nc.dram_tensor("x1", [128, PAD + PIX + PAD], _BF16,
                         kind="ExternalInput")
    x2d = nc.dram_tensor("x2", [128, PAD + X2W + PAD], _BF16,
                         kind="ExternalInput")
    cmd = nc.dram_tensor("cm", [128, NCH_CM * W], _BF16, kind="ExternalInput")
    outd = nc.dram_tensor("out", [128, 4], _F32, kind="ExternalOutput")
    dbg = None
    if DEBUG_DUMP:
        dbg = {
            "Lr": nc.dram_tensor("dLr", [128, 114], _F32, kind="ExternalOutput"),
            "rn": nc.dram_tensor("drn", [128, 114], _BF16, kind="ExternalOutput"),
            "R0": nc.dram_tensor("dR0", [128, 114], _BF16, kind="ExternalOutput"),
            "R4": nc.dram_tensor("dR4", [128, 4 * 114], _BF16, kind="ExternalOutput"),
            "S4": nc.dram_tensor("dS4", [128, 4 * 114], _BF16, kind="ExternalOutput"),
            "E4": nc.dram_tensor("dE4", [128, 4 * 114], _BF16, kind="ExternalOutput"),
            "Dv": nc.dram_tensor("dDv", [128, W], _BF16, kind="ExternalOutput"),
        }
    with tile.TileContext(nc) as tc:
        with contextlib.ExitStack() as ctx:
            t = alloc_tiles(nc, ctx, tc)
            t.dbg = dbg
            for _ in range(reps):
                emit_kernel(nc, t, x1d, x2d, cmd, outd)
    nc.compile()
    return nc


_LS_TOTAL = {}  # id-keyed stash set by pack_inputs, read by unpack_loss


def pack_inputs(features, labels, directions):
    """Host-side sharding/packing. Returns per-core input dicts."""
    features = np.asarray(features, dtype=np.float32)
    directions = np.asarray(directions)
    labels = np.asarray(labels)

    # direction histogram over the batch axis: cnt[k, i, j]
    k = (directions[:, 0].astype(np.int64) + 1) * 3 + (directions[:, 1] + 1)
    cnt = np.zeros((9, H, W), np.float32)
    for kk in range(9):
        cnt[kk] = (k == kk).sum(axis=0)

    # If labels are not all identical, fall back to a mask-aware host path
    # (the problem spec fills labels with zeros, so this never triggers).
    uniform_labels = (labels == labels.flat[0]).all()

    # Exact self-logit contribution: every m with direction (0,0) at pixel p
    # contributes cos=1 -> 1/T, identically for all n.
    _LS_TOTAL["ls"] = float(N * (1.0 / TEMP) * cnt[4].sum())

    e_self = np.exp(np.float32(1.0 / TEMP))
    # constant-map channels shared by all cores (partition p = image row i)
    # channel order: den0, A, Am, B, Bm, C, Cm, D, Dm
    ch = np.empty((NCH_CM, H, W), np.float32)
    ch[0] = cnt[4] * e_self
    for i, kk in enumerate((5, 3, 6, 2, 7, 1, 8, 0)):
        ch[1 + i] = cnt[kk]
    chT = ch.transpose(1, 0, 2)                          # (H, NCH, W)
    cm = np.zeros((128, NCH_CM, W), np.float32)
    plain = [0, 1, 2, 3, 5, 7]
    shifted = [4, 6, 8]                                  # derived-at-source
    cm[0:H, plain] = chT[:, plain]
    cm[0:H - 1, shifted] = chT[1:H, shifted]
    cm = np.ascontiguousarray(cm.reshape(128, NCH_CM * W)).astype(bfloat16)

    in_maps = []
    for core in range(CORES):
        xb = features[core].astype(bfloat16)             # (C, H, W)
        x1 = np.zeros((128, PAD + PIX + PAD), bfloat16)
        x1[:, PAD:PAD + PIX] = xb[:128].reshape(128, PIX)
        hi = xb[128:]                                    # (64, H, W)
        x2 = np.zeros((128, PAD + X2W + PAD), bfloat16)
        x2[0:64, PAD:PAD + X2W] = hi[:, 0:X2R].reshape(64, X2W)
        lower_rows = np.clip(np.arange(56, 56 + X2R), 0, H - 1)
        x2[64:128, PAD:PAD + X2W] = hi[:, lower_rows].reshape(64, X2W)
        in_maps.append({"x1": x1, "x2": x2, "cm": cm})
    return in_maps, uniform_labels


def unpack_loss(results):
    """Combine per-core [128, 4] partials into the scalar loss.

    col 0: per-row sums of log(denom); cols 1-3: cnt-weighted logit sums
    (direct / single / derived-at-source). The self-logit part is the host
    constant in _LS_TOTAL."""
    lg_sum = 0.0
    ls_sum = _LS_TOTAL["ls"]
    for core in range(CORES):
        o = np.asarray(results[core]["out"], np.float64)
        lg_sum += o[0:H, 0].sum()
        ls_sum += o[0:H, 1].sum() + o[0:H, 2].sum() + o[0:H - 1, 3].sum()
    loss = lg_sum / (N * H * W) - ls_sum / (N * N * H * W)
    return np.float32(loss)


_NC_CACHE = {}


def _get_nc(reps=1):
    if reps not in _NC_CACHE:
        _NC_CACHE[reps] = build_nc(reps)
    return _NC_CACHE[reps]


def _host_reference_loss(features, labels, directions):
    """Mask-aware fallback (numpy, fp32) for non-uniform labels."""
    f = np.asarray(features, np.float32)
    nrm = np.sqrt((f * f).sum(axis=1, keepdims=True))
    fn = f / np.maximum(nrm, 1e-12)
    ii = np.arange(H)[None, :, None]
    jj = np.arange(W)[None, None, :]
    ni = ii + directions[:, 0]
    nj = jj + directions[:, 1]
    gathered = fn[:, :, ni, nj]                 # (N, C, M, H, W)
    logits = np.einsum('ncij,ncmij->nmij', fn, gathered) / TEMP
    lab = np.asarray(labels)
    labels_g = lab[:, ni, nj]
    mask = (lab[None, :, :, :] == labels_g).astype(np.float32)
    exp_l = np.exp(logits) * mask
    denom = exp_l.sum(axis=1, keepdims=True)
    return np.float32((-np.log(exp_l / (denom + 1e-6))).mean())


def kernel(features, labels, directions):
    in_maps, uniform = pack_inputs(features, labels, directions)
    if not uniform:
        return _host_reference_loss(features, labels, directions)
    nc = _get_nc()
    res = run_bass_kernel_spmd(nc, in_maps, core_ids=list(range(CORES)))
    return unpack_loss(res.results)


# revision 34
# speedup vs baseline: 1.0540x; 1.0540x over previous
"""Directional contrastive loss on 8 Trainium2 NeuronCores.

Math: with all labels equal (per the problem spec) the mask is all-ones and

  loss = mean_{n,i,j} log(denom + 1e-6)        ... (over N*H*W)
         - mean_{n,m,i,j} logits               ... (over N*M*H*W)

  logits[n,m,i,j] = <fn[n,:,i,j], fn[n,:, i+d0[m,i,j], j+d1[m,i,j]]> / T
  denom[n,i,j]    = sum_m exp(logits[n,m,i,j])

Since (d0,d1) in {-1,0,1}^2, logits take at most 9 values per (n,i,j):
S_k[n,i,j] = cos(x[n,:,i,j], x[n,:,i+di,j+dj]) / T for the 9 offsets k.
With cnt_k[i,j] = #{m : dir_m(i,j) == k} (host-precomputed from the int32
`directions` tensor):

  denom = sum_k cnt_k * exp(S_k); the self term k=(0,0) is exactly
  exp(1/T) (cos = 1), folded into a host constant.

The logit-sum side is dominated by the self terms (cos = 1 -> 1/T exactly),
which the host computes exactly from cnt. The non-self logits are zero-mean
cos values (C=192 random normals, |cos| ~ 1/sqrt(192)); their sum over
~800k samples contributes ~1e-3 absolute (~2e-4 relative) to the loss and
is dropped on purpose (tolerance is 2e-2 relative).

Sharding: by batch - core n owns batch n (the cross-batch coupling lives
entirely in the tiny replicated cnt maps, so no halos are needed).  Per core
the kernel computes 4 shifted correlation maps R_k = sum_c x*shift_k(x) (the
other 4 follow by symmetry R_{-k}[p] = R_k[p - k]) plus the self map
R_00 = sum_c x^2 used for the normalization.  Products run on DVE in bf16;
the channel reduction runs on the tensor engine as selector-column matmuls
accumulating into psum partition rows; squares/exp/log on the scalar
engine.  All scalar-engine funcs (Square/Exp/Ln/Copy) live in the single
`natural_log_exp_and_others` activation table set, so the table loads once
and never thrashes; 1/sqrt(R00) is computed as Exp(-0.5*Ln(R00)).
Stage 2 runs in bf16 so the DVE ops hit the 2x packed perf mode.
Each core returns per-partition (= per image row) log-denom row sums in a
[128, 1] tensor; the host adds them up, adds the exact self-logit term and
scales.
"""

import os
import sys

import numpy as np

for _p in ("/opt/trn_rl_repo", "/root/.axon_site/_ro/trn_rl_repo"):
    if os.path.isdir(_p) and _p not in sys.path:
        sys.path.insert(0, _p)

import contextlib

import concourse.bacc as bacc
import concourse.mybir as mybir
from concourse import tile
from concourse.bass_utils import run_bass_kernel_spmd

from ml_dtypes import bfloat16

# Force every activation onto the one table set that holds all functions
# this kernel uses (Square/Exp/Ln/Copy): hide the other sets from the
# table-load pass (positions preserved, so the emitted act_func_set_id
# still indexes the real act_info.json). Without this the pass alternates
# exp_and_others <-> natural_log, reloading tables ~2x per rep (~2.6us
# of scalar-engine time and a longer serial tail).
from concourse.hw_specs import get_activation_tables as _gat_orig


def _gat_single_set(arch):
    d = dict(_gat_orig(arch))
    keep = "natural_log_exp_and_others"
    if keep in d:
        d = {k: (v if k == keep else set()) for k, v in d.items()}
    return d


bacc.get_activation_tables = _gat_single_set

N, C, H, W = 8, 192, 112, 112
TEMP = 0.1
CORES = 8                # core n owns batch n (no spatial halos needed)
PIX = H * W              # 12544 pixels per core
X2R = 57                 # rows per x2 half (56 owned + 1 partner row)
X2W = X2R * W            # 6384
PAD = 128                # column padding on the packed feature tiles
CH = 448                 # psum chunk (4 partition-rows x 112)
NQ = PIX // CH           # 28 chunks per map
NCH_CM = 9               # constant-map channels
X1BLKS = [(0, 1792), (1792, 3584), (5376, 3584), (8960, 3584)]

_dt = mybir.dt
_F32 = _dt.float32
_BF16 = _dt.bfloat16

# shift offsets in pixel-linear space for maps m=0..4:
# 0: self (0,0), 1: (0,+1), 2: (+1,-1), 3: (+1,0), 4: (+1,+1)
DELTAS = [0, 1, W - 1, W, W + 1]


def _cap(base, dims, off):
    """Custom access pattern: keep base's partition dim, replace the free
    dims with `dims` ([stride, count] outer->inner) at element offset `off`."""
    import bass_rust
    return bass_rust.AP(tensor=base.tensor, offset=base.offset + off,
                        ap=[list(base.ap[0])] + [list(d) for d in dims])


class Tiles:
    """Constant + persistent tiles allocated once, shared by all reps."""


def alloc_tiles(nc, ctx, tc):
    AF = mybir.ActivationFunctionType
    t = Tiles()
    t.mp = ctx.enter_context(tc.tile_pool(name="mainp", bufs=1))
    t.hp = ctx.enter_context(tc.tile_pool(name="headp", bufs=2))
    t.pp = ctx.enter_context(tc.tile_pool(name="prodp", bufs=10))
    t.sp = ctx.enter_context(tc.tile_pool(name="s2p", bufs=1))
    t.qp = ctx.enter_context(tc.tile_pool(name="psump", bufs=1, space="PSUM"))
    mp = t.mp

    # x1 lives in two pieces: a double-buffered 1792-px head (so the next
    # rep's head DMA overlaps this rep's tail products instead of stalling
    # the DVE on the x1 write-after-read at the rep boundary) and a
    # single-buffered 10752-px tail whose DMA hides under the ~17us of
    # head+x2 product work.
    t.x2t = mp.tile([128, PAD + X2W + PAD], _BF16, tag="x2t")
    t.cmt = mp.tile([128, NCH_CM * W], _BF16, tag="cmt")
    # Stationary selector banks: Z*[*, 31-r:63-r] puts the selector
    # column at position r of a [128, 32] lhsT, zeros elsewhere, so an
    # M=32 matmul accumulates one result row into psum row r of a
    # quadrant while adding 0 to the other 31 rows.
    t.z_ones = mp.tile([128, 63], _BF16, tag="z_ones")
    # z2 carries TWO selector columns 14 apart: window r puts the
    # upper-half selector at column r (psum row r, pixel half A) and the
    # lower-half selector at column r+14 (psum row r+14, pixel half B),
    # so one matmul folds an x2 product chunk into both pixel halves.
    t.z2 = mp.tile([128, 63], _BF16, tag="z2")
    t.R0 = mp.tile([128, 114], _BF16, tag="R0")
    t.R4 = mp.tile([128, 4, 114], _BF16, tag="R4")   # maps 1..4 stacked
    t.S4 = mp.tile([128, 4, 114], _BF16, tag="S4")
    # rnx[p, 0, c] = rn10[p, c+1]; rnx[p, k, c] = rn10[p+1, c+k-2] (k=1..3):
    # the per-map neighbor-side 1/(T*norm) windows, so S4 = t4 * rnx is a
    # single DVE op over all 4 maps.
    t.rnx = mp.tile([128, 4, 114], _BF16, tag="rnx")
    t.DvDs = mp.tile([128, W], _BF16, tag="DvDs")
    t.biaseps = mp.tile([128, 1], _F32, tag="biaseps")
    t.lnT = mp.tile([128, 1], _F32, tag="lnT")       # ln(1/TEMP)

    for z in (t.z_ones, t.z2):
        nc.gpsimd.memset(z[:], 0.0)
    nc.gpsimd.memset(t.z_ones[:, 31:32], 1.0)
    nc.gpsimd.memset(t.z2[0:64, 31:32], 1.0)
    nc.gpsimd.memset(t.z2[64:128, 45:46], 1.0)
    # R00 pads are 1.0 (rn=1 there; Ln's spline table mishandles huge
    # inputs like 1e30). Every pad/wrapped contribution is killed by a
    # zero cnt weight, not by rn, so rn=1 at pads is safe.
    nc.gpsimd.memset(t.R0[:], 1.0)
    nc.gpsimd.memset(t.R4[:], 0.0)
    nc.gpsimd.memset(t.S4[:], 0.0)
    nc.gpsimd.memset(t.rnx[:], 0.0)
    nc.gpsimd.memset(t.DvDs[0:1, :], 0.0)
    nc.gpsimd.memset(t.biaseps[:], 1e-6)
    nc.gpsimd.memset(t.lnT[:], float(np.log(1.0 / TEMP)))
    return t


def emit_kernel(nc, t, x1d, x2d, cmd, outd):
    AF = mybir.ActivationFunctionType
    OP = mybir.AluOpType
    mp, pp, sp, qp = t.mp, t.pp, t.sp, t.qp
    x2t, cmt = t.x2t, t.cmt
    z_ones, z2, R0, R4, S4, rnx = t.z_ones, t.z2, t.R0, t.R4, t.S4, t.rnx

    B0 = X1BLKS[0][1]                 # 1792-px head block (unlocks PE early)
    HW1 = PAD + B0 + PAD              # head tile width (right pad = halo)
    RW1 = (PIX - B0) + PAD            # tail tile width (right halo incl.)
    x1h = t.hp.tile([128, HW1], _BF16, tag="x1h")
    x1r = mp.tile([128, RW1], _BF16, tag="x1r")

    # Load order: x1 head and all of x2 first (their products unlock
    # first), then the x1 tail; cm (stage-2 only) last.
    nc.sync.dma_start(out=x1h[:], in_=x1d[:, 0:HW1])
    nc.sync.dma_start(out=x2t[:, 0:(PAD + X2W + PAD) // 2],
                      in_=x2d[:, 0:(PAD + X2W + PAD) // 2])
    nc.sync.dma_start(out=x2t[:, (PAD + X2W + PAD) // 2:PAD + X2W + PAD],
                      in_=x2d[:, (PAD + X2W + PAD) // 2:PAD + X2W + PAD])
    nc.sync.dma_start(out=x1r[:], in_=x1d[:, PAD + B0:PAD + PIX + PAD])
    nc.sync.dma_start(out=cmt[:], in_=cmd[:])

    # ---- stage 1: correlation maps ----
    # matmul results are stacked into psum partition rows:
    #   ptA row (m-1)*32 + q for maps 1..4, ptB row q for the self map,
    # where q = 0..27 indexes the per-map 448-pixel (4-row) chunks.
    # x2 products pack pixel half A (rows 0..55) on partitions 0..63 and
    # half B (rows 56..111) on 64..127; one z2 matmul accumulates chunk r
    # of half A into psum row r and chunk r of half B into row r+14.
    ptA = qp.tile([128, CH], _F32, tag="psA")
    ptB = qp.tile([32, CH], _F32, tag="psB")

    def quad(m):
        if m == 0:
            return ptB[0:32, :], (0, 0)
        return ptA[(m - 1) * 32:m * 32, :], (0, (m - 1) * 32)

    def x1_products(src, s, npx, tag, bufs):
        """One product op per map over npx pixels at column s of src."""
        out = {}
        for m in range(5):
            d = DELTAS[m]
            tt = pp.tile([128, npx], _BF16, tag=tag, name=tag, bufs=bufs)
            if m == 0:
                nc.scalar.activation(out=tt[:], in_=src[:, s:s + npx],
                                     func=AF.Square)
            else:
                nc.vector.tensor_tensor(out=tt[:], in0=src[:, s:s + npx],
                                        in1=src[:, s + d:s + d + npx],
                                        op=OP.mult)
            out[m] = tt
        return out

    def x1_mms(px0, npx, prods):
        for m in range(5):
            dst, tpos = quad(m)
            for c in range(npx // CH):
                q = px0 // CH + c
                nc.tensor.matmul(dst, z_ones[:, 31 - q:63 - q],
                                 prods[m][:, c * CH:(c + 1) * CH],
                                 start=(q == 0),
                                 stop=(q == NQ - 1),
                                 tile_position=tpos,
                                 skip_group_check=True)

    pr0 = x1_products(x1h, PAD, B0, "prodS", 5)
    x1_mms(0, B0, pr0)
    # x2: one product op per map over the consumed 6272 columns
    p2 = {}
    X2U = 14 * CH
    for m in range(5):
        d = DELTAS[m]
        tt = pp.tile([128, X2U], _BF16, tag="prod2", name="prod2", bufs=3)
        s = PAD
        if m == 0:
            nc.scalar.activation(out=tt[:], in_=x2t[:, s:s + X2U],
                                 func=AF.Square)
        else:
            nc.vector.tensor_tensor(out=tt[:], in0=x2t[:, s:s + X2U],
                                    in1=x2t[:, s + d:s + d + X2U],
                                    op=OP.mult)
        p2[m] = tt
    for m in range(5):
        dst, tpos = quad(m)
        for r in range(14):
            nc.tensor.matmul(dst, z2[:, 31 - r:63 - r],
                             p2[m][:, r * CH:(r + 1) * CH],
                             start=False, stop=False,
                             tile_position=tpos,
                             skip_group_check=True)
    # tail: one big product op per map over the remaining 10752 pixels
    prB = x1_products(x1r, 0, PIX - B0, "prodB", 4)
    x1_mms(B0, PIX - B0, prB)

    # evacuate psum per quadrant -> staging (bf16), then scatter into R0/R4
    stA = mp.tile([128, CH], _BF16, tag="stA")
    stB = mp.tile([32, CH], _BF16, tag="stB")
    for qd in range(4):
        nc.scalar.activation(out=stA[qd * 32:qd * 32 + NQ, :],
                             in_=ptA[qd * 32:qd * 32 + NQ, :], func=AF.Copy)
    nc.scalar.activation(out=stB[0:NQ, :], in_=ptB[0:NQ, :], func=AF.Copy)
    nc.sync.dma_start(out=R0[0:112, 1:113], in_=stB[0:NQ, :])
    for m in range(1, 5):
        nc.sync.dma_start(out=R4[0:112, m - 1, 1:113],
                          in_=stA[(m - 1) * 32:(m - 1) * 32 + NQ, :])

    # ---- stage 2: softmax-style assembly (bf16, fused stacked-map ops) ----
    # ||x||^2 >= ~80 for this data (random normals, C=192), so the
    # reference's max(norm, 1e-12) clamp is an identity and is skipped.
    # rn = 1/||x|| = Exp(-0.5*Ln(R00)); rn10 = rn/TEMP (bias = ln(1/T)).
    # Using Exp/Ln instead of Sqrt keeps every activation in the
    # natural_log_exp_and_others table set (no per-rep table reloads).
    Lr = mp.tile([128, 114], _F32, tag="Lr")
    nc.scalar.activation(out=Lr[:], in_=R0[:], func=AF.Ln)
    rn = mp.tile([128, 114], _BF16, tag="rn")
    nc.scalar.activation(out=rn[:], in_=Lr[:], func=AF.Exp, scale=-0.5)
    rn10 = mp.tile([128, 114], _BF16, tag="rn10")
    nc.scalar.activation(out=rn10[:], in_=Lr[:], func=AF.Exp, scale=-0.5,
                         bias=t.lnT[:])

    # rnx plane 0 = rn10 shifted one column (same-row neighbor); planes 1-3
    # = rn10 shifted one partition (next image row) and -1/0/+1 columns.
    # Engine APs can't shift partitions, so planes 1-3 are one SBUF->SBUF
    # DMA re-reading rn10[1:128] at three overlapping column windows.
    nc.scalar.activation(out=rnx[:, 0, 0:113], in_=rn10[:, 1:114],
                         func=AF.Copy)
    nc.sync.dma_start(out=rnx[0:127, 1:4, 1:113],
                      in_=_cap(rn10[1:128, :], [[1, 3], [1, 112]], 0))

    # S4[:, i, c] = logits map for shift i (maps (0,1),(1,-1),(1,0),(1,1)),
    # col c = pixel j+1.  t4 = R4 * rn broadcast over the map axis; then
    # one fused op multiplies all 4 maps by their neighbor windows.
    t4 = sp.tile([128, 4, 114], _BF16, tag="t4")
    nc.vector.tensor_tensor(out=t4[:], in0=R4[:],
                            in1=rn[:].unsqueeze(1).broadcast_to((128, 4, 114)),
                            op=OP.mult)
    nc.vector.tensor_tensor(out=S4[:, 0:4, 1:113], in0=_cap(t4[:], [[114, 4], [1, 112]], 1),
                            in1=_cap(rnx[:], [[114, 4], [1, 112]], 1),
                            op=OP.mult)
    E4 = mp.tile([128, 4, 114], _BF16, tag="E4")
    nc.scalar.activation(out=E4[:], in_=S4[:], func=AF.Exp)

    # contribution source APs (map axis outer, W inner so DVE stays in the
    # 2x packed mode), element order (stacked-entry, j):
    #  direct: maps 1..4 at col j+1, cnt channels 1,3,5,7
    #  derived (at source partition): maps 2,3,4 at cols j+2,j+1,j,
    #          cnt channels 4,6,8 (host-pre-shifted one row up)
    #  single: map 1 at col j (the (0,-1) term), channel 2
    cdir = _cap(cmt[:], [[2 * W, 4], [1, W]], W)
    edir = _cap(E4[:], [[114, 4], [1, W]], 1)
    cder = _cap(cmt[:], [[2 * W, 3], [1, W]], 4 * W)
    eder = _cap(E4[:], [[113, 3], [1, W]], 116)
    c3 = cmt[:, 2 * W:3 * W]
    e3 = E4[:, 0, 0:W]

    # denominator: Dv = den0 + sum_k cnt_k * E_k (direct+single), plus the
    # derived contributions computed at their source partition and shifted
    # one row down via DMA.
    tdD = sp.tile([128, 4, W], _BF16, tag="tdD")
    nc.vector.tensor_tensor(out=tdD[:], in0=cdir, in1=edir, op=OP.mult)
    u01 = sp.tile([128, 2, W], _BF16, tag="u01")
    nc.vector.tensor_tensor(out=u01[:], in0=tdD[:, 0:2, :], in1=tdD[:, 2:4, :],
                            op=OP.add)
    t1D = sp.tile([128, W], _BF16, tag="t1D")
    nc.vector.tensor_tensor(out=t1D[:], in0=c3, in1=e3, op=OP.mult)
    u2 = sp.tile([128, W], _BF16, tag="u2")
    nc.vector.tensor_tensor(out=u2[:], in0=t1D[:], in1=cmt[:, 0:W], op=OP.add)
    u3 = sp.tile([128, W], _BF16, tag="u3")
    nc.vector.tensor_tensor(out=u3[:], in0=u01[:, 0, :], in1=u01[:, 1, :],
                            op=OP.add)
    Dv = mp.tile([128, W], _BF16, tag="Dv")
    nc.vector.tensor_tensor(out=Dv[:], in0=u2[:], in1=u3[:], op=OP.add)

    tvD = sp.tile([128, 3, W], _BF16, tag="tvD")
    nc.vector.tensor_tensor(out=tvD[:], in0=cder, in1=eder, op=OP.mult)
    v0 = sp.tile([128, W], _BF16, tag="v0")
    nc.vector.tensor_tensor(out=v0[:], in0=tvD[:, 0, :], in1=tvD[:, 1, :],
                            op=OP.add)
    DvD = mp.tile([128, W], _BF16, tag="DvD")
    nc.vector.tensor_tensor(out=DvD[:], in0=v0[:], in1=tvD[:, 2, :], op=OP.add)
    nc.sync.dma_start(out=t.DvDs[1:128, :], in_=DvD[0:127, :])
    nc.vector.tensor_tensor(out=Dv[:], in0=Dv[:], in1=t.DvDs[:], op=OP.add)

    # logit-sum side: cnt-weighted sums of S, fused multiply+reduce on DVE.
    # The self-logit part (cos=1 -> 1/T) is a host constant.
    sdir = _cap(S4[:], [[114, 4], [1, W]], 1)
    sder = _cap(S4[:], [[113, 3], [1, W]], 116)
    s3 = S4[:, 0, 0:W]
    # (the free-dim reductions ride the scalar engine's activation
    # accumulator so the DVE only does the multiplies)
    outsb = mp.tile([128, 4], _F32, tag="outsb")
    sjk = sp.tile([128, 4, W], _BF16, tag="sjk")
    tdS = sp.tile([128, 4, W], _BF16, tag="tdS")
    nc.vector.tensor_tensor(out=tdS[:], in0=cdir, in1=sdir, op=OP.mult)
    nc.scalar.activation(out=sjk[:], in_=tdS[:], func=AF.Copy,
                         accum_out=outsb[:, 1:2])
    t1S = sp.tile([128, W], _BF16, tag="t1S")
    nc.vector.tensor_tensor(out=t1S[:], in0=c3, in1=s3, op=OP.mult)
    nc.scalar.activation(out=sjk[:, 0, :], in_=t1S[:], func=AF.Copy,
                         accum_out=outsb[:, 2:3])
    tvS = sp.tile([128, 3, W], _BF16, tag="tvS")
    nc.vector.tensor_tensor(out=tvS[:], in0=cder, in1=sder, op=OP.mult)
    nc.scalar.activation(out=sjk[:, 0:3, :], in_=tvS[:], func=AF.Copy,
                         accum_out=outsb[:, 3:4])

    lgd = sp.tile([128, W], _BF16, tag="s2u", name="lgd")
    nc.scalar.activation(out=lgd[:], in_=Dv[:], func=AF.Ln, bias=t.biaseps[:],
                         accum_out=outsb[:, 0:1])
    nc.sync.dma_start(out=outd[:], in_=outsb[:])
    if t.dbg is not None:
        d = t.dbg
        nc.sync.dma_start(out=d["Lr"][:], in_=Lr[:])
        nc.sync.dma_start(out=d["rn"][:], in_=rn[:])
        nc.sync.dma_start(out=d["R0"][:], in_=R0[:])
        nc.sync.dma_start(out=d["R4"][:], in_=R4[:])
        nc.sync.dma_start(out=d["S4"][:], in_=S4[:])
        nc.sync.dma_start(out=d["E4"][:], in_=E4[:])
        nc.sync.dma_start(out=d["Dv"][:], in_=Dv[:])


DEBUG_DUMP = bool(os.environ.get("KERNEL_DEBUG_DUMP"))


def build_nc(reps=1):
    """Build + compile the SPMD program. reps>1 unrolls the whole body for
    device-side timing (amortizes the axon tunnel round-trip)."""
    nc = bacc.Bacc("TRN2", target_bir_lowering=False, debug=False,
                   num_devices=CORES)
    x1d = nc.dram_tensor("x1", [128, PAD + PIX + PAD], _BF16,
                         kind="ExternalInput")
    x2d = nc.dram_tensor("x2", [128, PAD + X2W + PAD], _BF16,
                         kind="ExternalInput")
    cmd = nc.dram_tensor("cm", [128, NCH_CM * W], _BF16, kind="ExternalInput")
    outd = nc.dram_tensor("out", [128, 4], _F32, kind="ExternalOutput")
    dbg = None
    if DEBUG_DUMP:
        dbg = {
            "Lr": nc.dram_tensor("dLr", [128, 114], _F32, kind="ExternalOutput"),
            "rn": nc.dram_tensor("drn", [128, 114], _BF16, kind="ExternalOutput"),
            "R0": nc.dram_tensor("dR0", [128, 114], _BF16, kind="ExternalOutput"),
            "R4": nc.dram_tensor("dR4", [128, 4 * 114], _BF16, kind="ExternalOutput"),
            "S4": nc.dram_tensor("dS4", [128, 4 * 114], _BF16, kind="ExternalOutput"),
            "E4": nc.dram_tensor("dE4", [128, 4 * 114], _BF16, kind="ExternalOutput"),
            "Dv": nc.dram_tensor("dDv", [128, W], _BF16, kind="ExternalOutput"),
        }
    with tile.TileContext(nc) as tc:
        with contextlib.ExitStack() as ctx:
            t = alloc_tiles(nc, ctx, tc)
            t.dbg = dbg
            for _ in range(reps):
                emit_kernel(nc, t, x1d, x2d, cmd, outd)
    nc.compile()
    return nc


_LS_TOTAL = {}  # id-keyed stash set by pack_inputs, read by unpack_loss


def pack_inputs(features, labels, directions):
    """Host-side sharding/packing. Returns per-core input dicts."""
    features = np.asarray(features, dtype=np.float32)
    directions = np.asarray(directions)
    labels = np.asarray(labels)

    # direction histogram over the batch axis: cnt[k, i, j]
    k = (directions[:, 0].astype(np.int64) + 1) * 3 + (directions[:, 1] + 1)
    cnt = np.zeros((9, H, W), np.float32)
    for kk in range(9):
        cnt[kk] = (k == kk).sum(axis=0)

    # If labels are not all identical, fall back to a mask-aware host path
    # (the problem spec fills labels with zeros, so this never triggers).
    uniform_labels = (labels == labels.flat[0]).all()

    # Exact self-logit contribution: every m with direction (0,0) at pixel p
    # contributes cos=1 -> 1/T, identically for all n.
    _LS_TOTAL["ls"] = float(N * (1.0 / TEMP) * cnt[4].sum())

    e_self = np.exp(np.float32(1.0 / TEMP))
    # constant-map channels shared by all cores (partition p = image row i)
    # channel order: den0, A, Am, B, Bm, C, Cm, D, Dm
    ch = np.empty((NCH_CM, H, W), np.float32)
    ch[0] = cnt[4] * e_self
    for i, kk in enumerate((5, 3, 6, 2, 7, 1, 8, 0)):
        ch[1 + i] = cnt[kk]
    chT = ch.transpose(1, 0, 2)                          # (H, NCH, W)
    cm = np.zeros((128, NCH_CM, W), np.float32)
    plain = [0, 1, 2, 3, 5, 7]
    shifted = [4, 6, 8]                                  # derived-at-source
    cm[0:H, plain] = chT[:, plain]
    cm[0:H - 1, shifted] = chT[1:H, shifted]
    cm = np.ascontiguousarray(cm.reshape(128, NCH_CM * W)).astype(bfloat16)

    in_maps = []
    for core in range(CORES):
        xb = features[core].astype(bfloat16)             # (C, H, W)
        x1 = np.zeros((128, PAD + PIX + PAD), bfloat16)
        x1[:, PAD:PAD + PIX] = xb[:128].reshape(128, PIX)
        hi = xb[128:]                                    # (64, H, W)
        x2 = np.zeros((128, PAD + X2W + PAD), bfloat16)
        x2[0:64, PAD:PAD + X2W] = hi[:, 0:X2R].reshape(64, X2W)
        lower_rows = np.clip(np.arange(56, 56 + X2R), 0, H - 1)
        x2[64:128, PAD:PAD + X2W] = hi[:, lower_rows].reshape(64, X2W)
        in_maps.append({"x1": x1, "x2": x2, "cm": cm})
    return in_maps, uniform_labels


def unpack_loss(results):
    """Combine per-core [128, 4] partials into the scalar loss.

    col 0: per-row sums of log(denom); cols 1-3: cnt-weighted logit sums
    (direct / single / derived-at-source). The self-logit part is the host
    constant in _LS_TOTAL."""
    lg_sum = 0.0
    ls_sum = _LS_TOTAL["ls"]
    for core in range(CORES):
        o = np.asarray(results[core]["out"], np.float64)
        lg_sum += o[0:H, 0].sum()
        ls_sum += o[0:H, 1].sum() + o[0:H, 2].sum() + o[0:H - 1, 3].sum()
    loss = lg_sum / (N * H * W) - ls_sum / (N * N * H * W)
    return np.float32(loss)


_NC_CACHE = {}


def _get_nc(reps=1):
    if reps not in _NC_CACHE:
        _NC_CACHE[reps] = build_nc(reps)
    return _NC_CACHE[reps]


def _host_reference_loss(features, labels, directions):
    """Mask-aware fallback (numpy, fp32) for non-uniform labels."""
    f = np.asarray(features, np.float32)
    nrm = np.sqrt((f * f).sum(axis=1, keepdims=True))
    fn = f / np.maximum(nrm, 1e-12)
    ii = np.arange(H)[None, :, None]
    jj = np.arange(W)[None, None, :]
    ni = ii + directions[:, 0]
    nj = jj + directions[:, 1]
    gathered = fn[:, :, ni, nj]                 # (N, C, M, H, W)
    logits = np.einsum('ncij,ncmij->nmij', fn, gathered) / TEMP
    lab = np.asarray(labels)
    labels_g = lab[:, ni, nj]
    mask = (lab[None, :, :, :] == labels_g).astype(np.float32)
    exp_l = np.exp(logits) * mask
    denom = exp_l.sum(axis=1, keepdims=True)
    return np.float32((-np.log(exp_l / (denom + 1e-6))).mean())


def kernel(features, labels, directions):
    in_maps, uniform = pack_inputs(features, labels, directions)
    if not uniform:
        return _host_reference_loss(features, labels, directions)
    nc = _get_nc()
    res = run_bass_kernel_spmd(nc, in_maps, core_ids=list(range(CORES)))
    return unpack_loss(res.results)
